# revision 17
# baseline (speedup 1.0000x reference)
"""CJS loss kernel for Trainium2 (8 NeuronCores, SPMD data-parallel).

loss = sum_{b,j} w_j * e[b,j] / B     with w_j = C - j and
e = 0.5*(D*(logD - logm) + Dp*(logDp - logm)),  m = 0.5*(D + Dp + EPS)

Sharding: batch dim B=16384 split into 8 shards of 2048 rows; each core
reduces its shard to per-column sums colsum[j] = sum_b e[b,j] (output
[1, 2048] per core); the host finishes with the tiny weighted dot and
the divide by B.

Per 128-row tile on each core (default config):
  ACT :  logD = Ln(D), logDp = Ln(Dp), logm = Ln(0.5*s + EPS/2) from PSUM
  PE  :  s = identity@D + identity@Dp   (f32r matmuls into PSUM)
         colsum += 0.5*ones@p1 + 0.5*ones@p2 - 0.5*ones@p3  (PSUM
         accumulation across all 16 row tiles; the reduction matmuls for
         tile k are emitted in iteration k+1 so the next tile's identity
         matmuls always lead on the in-order PE queue)
  DVE :  p1 = D*logD, p3 = s*logm
  POOL:  p2 = Dp*logDp   (GpSimd, ~2x slower/elem but runs in parallel)

Every engine lands under the per-core DMA roofline (33.5 MB / 358 GB/s
= 93.5 us): cost-model busy times are ACT 93.8, DMA 93.5, DVE 75.9,
Pool 66.9, PE 69.2 us; predicted wall 118 us (pipeline ramp + Tile
drain/barrier tail account for the rest).

float32r (TF32-like single-pass PE mode, 1 cycle/row vs 4 for fp32) is
used for every matmul operand. The BIR verifier demands f32r-rounded
producers, so the DRAM params and input tiles are declared f32r (numpy
f32 bits feed them directly; PE rounds internally) and the DVE/Pool
product tiles are written as f32r. Measured end-to-end relative error
on hardware: 9.3e-6.

Non-default build_kernel() options (split_sbuf / four_prod / acc_onebank
variants) are kept for reference; they all simmed slower.
"""

import sys

sys.path.insert(0, "/opt/trn_rl_repo")

import numpy as np

import concourse.bass as bass
import concourse.mybir as mybir
from concourse import tile
from concourse.bass_utils import run_bass_kernel_spmd

N_CORES = 8
B, C = 16384, 2048
B_SHARD = B // N_CORES          # 2048 rows per core
ROW_TILES = B_SHARD // 128      # 16 tiles of 128 rows
EPS = 1e-8

F32 = mybir.dt.float32
F32R = mybir.dt.float32r
LN = mybir.ActivationFunctionType.Ln
MULT = mybir.AluOpType.mult
ADD = mybir.AluOpType.add


def _fix_multi_waits(nc):
    """This walrus build accepts only one sync-wait per instruction, while
    Tile's sem-assigner can attach several. Split extras onto single-wait
    EventSemaphore carriers inserted before the instruction (same engine,
    identical blocking semantics)."""
    for fn in nc.m.functions:
        for blk in fn.blocks:
            insns = list(blk.instructions)
            out = []
            changed = False
            for ins in insns:
                si = ins.sync_info
                waits = list(si.on_wait) if si is not None else []
                if len(waits) > 1:
                    for k, w in enumerate(waits[:-1]):
                        carrier = mybir.InstEventSemaphore(
                            name=f"{ins.name}-wsplit{k}",
                            engine=ins.engine,
                            ins=[],
                            outs=[],
                            sync_info=mybir.SyncInfo(on_wait=[w], on_update=[]),
                        )
                        carrier.debug = ins.debug
                        nc.register_instruction(carrier)
                        out.append(carrier)
                    ins.sync_info = mybir.SyncInfo(
                        on_wait=[waits[-1]], on_update=list(si.on_update)
                    )
                    changed = True
                out.append(ins)
            if changed:
                blk.instructions.clear()
                blk.instructions.extend(out)


def build_kernel(p2_engine="gpsimd", inp_bufs=3, dp_bufs=None, logs_bufs=2, prods_bufs=3, halfp_bufs=4, ps_s_bufs=2, dma_split=2, pipelined_reds=True, acc_onebank=False,
                 s_mode="pe_psum", s_pool_cols=1024, dma_engs=("sync", "sync"),
                 _hack_acc=False, p3b_pool_cols=1248):
    nc = bass.Bass("TRN2", target_bir_lowering=False, debug=False,
                   num_devices=N_CORES)

    d_ext = nc.dram_tensor("D", [B_SHARD, C], F32R, kind="ExternalInput").ap()
    dp_ext = nc.dram_tensor("Dp", [B_SHARD, C], F32R, kind="ExternalInput").ap()
    ident_ext = nc.dram_tensor("ident", [128, 128], F32R, kind="ExternalInput").ap()
    whalf_ext = nc.dram_tensor("whalf", [128, 1], F32R, kind="ExternalInput").ap()
    wneg_ext = nc.dram_tensor("wneg", [128, 1], F32R, kind="ExternalInput").ap()
    out_ext = nc.dram_tensor("colsum", [1, C], F32, kind="ExternalOutput").ap()

    with tile.TileContext(nc) as tc:
        with tc.tile_pool(name="consts", bufs=1) as consts, \
             tc.tile_pool(name="inp", bufs=inp_bufs) as inp, \
             tc.tile_pool(name="logs", bufs=logs_bufs) as logs, \
             tc.tile_pool(name="prods", bufs=prods_bufs) as prods, \
             tc.tile_pool(name="half", bufs=halfp_bufs) as halfp, \
             tc.tile_pool(name="ps_s", bufs=ps_s_bufs, space="PSUM") as ps_s, \
             tc.tile_pool(name="ps_acc", bufs=1, space="PSUM") as ps_acc:

            ident = consts.tile([128, 128], F32R)
            nc.sync.dma_start(ident[:], ident_ext[:])
            whalf = consts.tile([128, 1], F32R)
            nc.sync.dma_start(whalf[:], whalf_ext[:])
            wneg = consts.tile([128, 1], F32R)
            nc.sync.dma_start(wneg[:], wneg_ext[:])
            bias_eps = consts.tile([128, 1], F32)
            nc.vector.memset(bias_eps[:], EPS / 2)

            if _hack_acc:
                acc = ps_acc.tile([1, 512], F32)
            elif acc_onebank:
                acc = ps_acc.tile([128, 512], F32)
            else:
                acc = ps_acc.tile([1, C], F32)

            def emit_reds(p1, p2, p3s, first, last):
                if isinstance(p3s, tuple) and p3s and p3s[0] == "four":
                    _, p3a, p3b = p3s
                    for cc in range(4):
                        cs = slice(cc * 512, (cc + 1) * 512)
                        nc.tensor.matmul(acc[:, cs], whalf[:], p1[:, cs],
                                         start=first, stop=False)
                        nc.tensor.matmul(acc[:, cs], whalf[:], p2[:, cs],
                                         start=False, stop=False)
                        nc.tensor.matmul(acc[:, cs], wneg[:], p3a[:, cs],
                                         start=False, stop=False)
                        nc.tensor.matmul(acc[:, cs], wneg[:], p3b[:, cs],
                                         start=False, stop=last)
                    return
                for h in range(2):
                    p3 = p3s[h]
                    for q in range(2):
                        cc = 2 * h + q
                        cs = slice(h * 1024 + q * 512, h * 1024 + (q + 1) * 512)
                        ps = slice(q * 512, (q + 1) * 512)
                        if _hack_acc:
                            acc_out = acc[:, :]
                            tp = None
                        elif acc_onebank:
                            acc_out = acc[32 * cc:32 * cc + 1, :]
                            tp = (0, 32 * cc)
                        else:
                            acc_out = acc[:, cs]
                            tp = None
                        nc.tensor.matmul(acc_out, whalf[:], p1[:, cs],
                                         start=first, stop=False,
                                         tile_position=tp)
                        nc.tensor.matmul(acc_out, whalf[:], p2[:, cs],
                                         start=False, stop=False,
                                         tile_position=tp)
                        nc.tensor.matmul(acc_out, wneg[:], p3[:, ps],
                                         start=False, stop=last,
                                         tile_position=tp)

            pending = None
            for rt in range(ROW_TILES):
                rows = slice(rt * 128, (rt + 1) * 128)
                d_t = inp.tile([128, C], F32R, tag="d")
                dp_t = inp.tile([128, C], F32R, tag="dp", bufs=dp_bufs or inp_bufs)
                cw = C // dma_split
                eng_d = getattr(nc, {"sync": "sync", "scalar": "scalar",
                                     "vector": "vector"}[dma_engs[0]])
                eng_dp = getattr(nc, {"sync": "sync", "scalar": "scalar",
                                      "vector": "vector"}[dma_engs[1]])
                for k in range(dma_split):
                    sl = slice(k * cw, (k + 1) * cw)
                    eng_d.dma_start(d_t[:, sl], d_ext[rows, sl])
                    eng_dp.dma_start(dp_t[:, sl], dp_ext[rows, sl])

                log_d = logs.tile([128, C], F32, tag="logd")
                nc.scalar.activation(log_d[:], d_t[:].bitcast(F32), LN)
                log_dp = logs.tile([128, C], F32, tag="logdp")
                nc.scalar.activation(log_dp[:], dp_t[:].bitcast(F32), LN)

                p1 = prods.tile([128, C], F32R, tag="p1")
                nc.vector.tensor_tensor(p1[:], d_t[:].bitcast(F32), log_d[:], MULT)
                p2 = prods.tile([128, C], F32R, tag="p2")
                p2_eng = nc.gpsimd if p2_engine == "gpsimd" else nc.vector
                p2_eng.tensor_tensor(p2[:], dp_t[:].bitcast(F32), log_dp[:], MULT)

                if s_mode == "four_prod":
                    logm = halfp.tile([128, C], F32, tag="logm")
                    for h in range(2):
                        s_ps = ps_s.tile([128, 1024], F32, tag="s")
                        for q in range(2):
                            cs = slice(h * 1024 + q * 512,
                                       h * 1024 + (q + 1) * 512)
                            ps = slice(q * 512, (q + 1) * 512)
                            nc.tensor.matmul(s_ps[:, ps], ident[:], d_t[:, cs],
                                             start=True, stop=False)
                            nc.tensor.matmul(s_ps[:, ps], ident[:],
                                             dp_t[:, cs], start=False,
                                             stop=True)
                        nc.scalar.activation(logm[:, h * 1024:(h + 1) * 1024],
                                             s_ps[:], LN,
                                             scale=0.5, bias=bias_eps[:])
                    p3a = prods.tile([128, C], F32R, tag="p3a")
                    nc.vector.tensor_tensor(p3a[:], d_t[:].bitcast(F32),
                                            logm[:], MULT)
                    p3b = prods.tile([128, C], F32R, tag="p3b")
                    pc = p3b_pool_cols
                    if pc > 0:
                        nc.gpsimd.tensor_tensor(p3b[:, :pc],
                                                dp_t[:, :pc].bitcast(F32),
                                                logm[:, :pc], MULT)
                    if pc < C:
                        nc.vector.tensor_tensor(p3b[:, pc:],
                                                dp_t[:, pc:].bitcast(F32),
                                                logm[:, pc:], MULT)
                    p3s = ("four", p3a, p3b)
                elif s_mode == "split_sbuf":
                    s_t = halfp.tile([128, C], F32, tag="s")
                    pc = s_pool_cols
                    if pc > 0:
                        nc.gpsimd.tensor_tensor(
                            s_t[:, :pc], d_t[:, :pc].bitcast(F32),
                            dp_t[:, :pc].bitcast(F32), ADD)
                    if pc < C:
                        nc.vector.tensor_tensor(
                            s_t[:, pc:], d_t[:, pc:].bitcast(F32),
                            dp_t[:, pc:].bitcast(F32), ADD)
                    logm = halfp.tile([128, C], F32, tag="logm")
                    nc.scalar.activation(logm[:], s_t[:], LN,
                                         scale=0.5, bias=bias_eps[:])
                    p3 = halfp.tile([128, C], F32R, tag="p3")
                    nc.vector.tensor_tensor(p3[:], s_t[:], logm[:], MULT)
                    p3s = [p3[:, :1024], p3[:, 1024:]]
                else:
                    p3s = []
                    for h in range(2):
                        s_ps = ps_s.tile([128, 1024], F32, tag="s")
                        for q in range(2):
                            cs = slice(h * 1024 + q * 512,
                                       h * 1024 + (q + 1) * 512)
                            ps = slice(q * 512, (q + 1) * 512)
                            nc.tensor.matmul(s_ps[:, ps], ident[:], d_t[:, cs],
                                             start=True, stop=False)
                            nc.tensor.matmul(s_ps[:, ps], ident[:],
                                             dp_t[:, cs], start=False,
                                             stop=True)

                        logm = halfp.tile([128, 1024], F32, tag="logm")
                        nc.scalar.activation(logm[:], s_ps[:], LN,
                                             scale=0.5, bias=bias_eps[:])
                        p3 = halfp.tile([128, 1024], F32R, tag="p3")
                        nc.vector.tensor_tensor(p3[:], s_ps[:], logm[:], MULT)
                        p3s.append(p3)

                if not pipelined_reds:
                    emit_reds(p1, p2, p3s, rt == 0, rt == ROW_TILES - 1)
                else:
                    if pending is not None:
                        emit_reds(pending[0], pending[1], pending[2],
                                  pending[3], False)
                    pending = (p1, p2, p3s, rt == 0)
            if pipelined_reds and pending is not None:
                emit_reds(pending[0], pending[1], pending[2],
                          pending[3], True)

            if acc_onebank:
                out_sb = consts.tile([128, 512], F32)
                for cc in range(4):
                    pr = slice(32 * cc, 32 * cc + 1)
                    nc.vector.tensor_copy(out_sb[pr, :], acc[pr, :])
                    nc.sync.dma_start(
                        out_ext[0:1, cc * 512:(cc + 1) * 512], out_sb[pr, :])
            else:
                out_sb = consts.tile([1, C], F32)
                for q in range(4):
                    ps = slice(q * 512, (q + 1) * 512)
                    src_ps = slice(0, 512) if _hack_acc else ps
                    nc.vector.tensor_copy(out_sb[:, ps], acc[:, src_ps])
                nc.sync.dma_start(out_ext[:], out_sb[:])

    _fix_multi_waits(nc)
    return nc


_NC_CACHE = None


def _get_nc():
    global _NC_CACHE
    if _NC_CACHE is None:
        _NC_CACHE = build_kernel()
    return _NC_CACHE


def make_in_maps(D, D_pred):
    D = np.ascontiguousarray(np.asarray(D, dtype=np.float32))
    D_pred = np.ascontiguousarray(np.asarray(D_pred, dtype=np.float32))
    ident = np.eye(128, dtype=np.float32)
    whalf = np.full((128, 1), 0.5, dtype=np.float32)
    wneg = np.full((128, 1), -0.5, dtype=np.float32)
    in_maps = []
    for c in range(N_CORES):
        rows = slice(c * B_SHARD, (c + 1) * B_SHARD)
        in_maps.append({
            "D": D[rows],
            "Dp": D_pred[rows],
            "ident": ident,
            "whalf": whalf,
            "wneg": wneg,
        })
    return in_maps


def finish(results):
    colsum = np.zeros(C, dtype=np.float64)
    for c in range(N_CORES):
        colsum += results[c]["colsum"][0].astype(np.float64)
    w = (C - np.arange(C)).astype(np.float64)
    return np.asarray((colsum * w).sum() / B, dtype=np.float32)


def kernel(D, D_pred):
    nc = _get_nc()
    res = run_bass_kernel_spmd(nc, make_in_maps(D, D_pred), list(range(N_CORES)))
    return finish(res.results)


if __name__ == "__main__":
    rng = np.random.default_rng(0)
    D = rng.uniform(1e-4, 1.0, (B, C)).astype(np.float32)
    Dp = rng.uniform(1e-4, 1.0, (B, C)).astype(np.float32)
    out = kernel(D, Dp)
    print("kernel out:", out)


# revision 22
# speedup vs baseline: 1.0144x; 1.0144x over previous
"""CJS loss kernel for Trainium2 (8 NeuronCores, SPMD data-parallel).

loss = sum_{b,j} w_j * e[b,j] / B     with w_j = C - j and
e = 0.5*(D*(logD - logm) + Dp*(logDp - logm)),  m = 0.5*(D + Dp + EPS)

Sharding: batch dim B=16384 split into 8 shards of 2048 rows; each core
reduces its shard to per-column sums colsum[j] = sum_b e[b,j] (output
[1, 2048] per core); the host finishes with the tiny weighted dot and
the divide by B.

Per 128-row tile on each core (default config):
  ACT :  logD = Ln(D), logDp = Ln(Dp), logm = Ln(0.5*s + EPS/2) from PSUM
  PE  :  s = identity@D + identity@Dp   (f32r matmuls into PSUM)
         colsum += 0.5*ones@p1 + 0.5*ones@p2 - 0.5*ones@p3  (PSUM
         accumulation across all 16 row tiles; the reduction matmuls for
         tile k are emitted in iteration k+1 so the next tile's identity
         matmuls always lead on the in-order PE queue)
  DVE :  p1 = D*logD, p3 = s*logm
  POOL:  p2 = Dp*logDp   (GpSimd, ~2x slower/elem but runs in parallel)

Every engine lands under the per-core DMA roofline (33.5 MB / 358 GB/s
= 93.5 us): cost-model busy times are ACT 93.8, DMA 93.5, DVE 75.9,
Pool 66.9, PE 69.2 us; predicted wall 118 us (pipeline ramp + Tile
drain/barrier tail account for the rest).

float32r (TF32-like single-pass PE mode, 1 cycle/row vs 4 for fp32) is
used for every matmul operand. The BIR verifier demands f32r-rounded
producers, so the DRAM params and input tiles are declared f32r (numpy
f32 bits feed them directly; PE rounds internally) and the DVE/Pool
product tiles are written as f32r. Measured end-to-end relative error
on hardware: 9.3e-6.

Non-default build_kernel() options (split_sbuf / four_prod / acc_onebank
variants) are kept for reference; they all simmed slower.
"""

import sys

sys.path.insert(0, "/opt/trn_rl_repo")

import numpy as np

import concourse.bass as bass
import concourse.mybir as mybir
from concourse import tile
from concourse.bass_utils import run_bass_kernel_spmd

N_CORES = 8
B, C = 16384, 2048
B_SHARD = B // N_CORES          # 2048 rows per core
ROW_TILES = B_SHARD // 128      # 16 tiles of 128 rows
EPS = 1e-8

F32 = mybir.dt.float32
F32R = mybir.dt.float32r
LN = mybir.ActivationFunctionType.Ln
MULT = mybir.AluOpType.mult
ADD = mybir.AluOpType.add


def _fix_multi_waits(nc):
    """This walrus build accepts only one sync-wait per instruction, while
    Tile's sem-assigner can attach several. Split extras onto single-wait
    EventSemaphore carriers inserted before the instruction (same engine,
    identical blocking semantics)."""
    for fn in nc.m.functions:
        for blk in fn.blocks:
            insns = list(blk.instructions)
            out = []
            changed = False
            for ins in insns:
                si = ins.sync_info
                waits = list(si.on_wait) if si is not None else []
                if len(waits) > 1:
                    for k, w in enumerate(waits[:-1]):
                        carrier = mybir.InstEventSemaphore(
                            name=f"{ins.name}-wsplit{k}",
                            engine=ins.engine,
                            ins=[],
                            outs=[],
                            sync_info=mybir.SyncInfo(on_wait=[w], on_update=[]),
                        )
                        carrier.debug = ins.debug
                        nc.register_instruction(carrier)
                        out.append(carrier)
                    ins.sync_info = mybir.SyncInfo(
                        on_wait=[waits[-1]], on_update=list(si.on_update)
                    )
                    changed = True
                out.append(ins)
            if changed:
                blk.instructions.clear()
                blk.instructions.extend(out)


class _SlimTailTileContext(tile.TileContext):
    """Skip the second all-engine barrier of the stock kernel tail (the one
    after clear_and_free_semaphores). The final drain already waited for all
    outstanding work and the first barrier aligned the engines; the second
    only guards sem-clears against an engine racing ahead into... nothing
    (end of program). Validated by repeat executions on hardware."""

    def _drain_and_barrier(self, tick_clock, wait_clock):
        from concourse.vector_clock import ScopedClock
        drain_inst = self.nc.sync.drain()
        wait_clock.add_sem_waits(
            drain_inst.ins, ScopedClock({None: tick_clock.global_clock})
        )
        self.nc.all_engine_barrier()
        assert self.sems is not None
        popped = self.nc._tile_sem_poison_stack.pop()
        assert popped is self._sem_poison
        self.nc.clear_and_free_semaphores(list(self.sems.allocated().values()))


def build_kernel(p2_engine="gpsimd", inp_bufs=3, dp_bufs=None, logs_bufs=2, prods_bufs=3, halfp_bufs=4, ps_s_bufs=2, dma_split=2, pipelined_reds=True, acc_onebank=False,
                 s_mode="pe_psum", s_pool_cols=1024, dma_engs=("sync", "sync"),
                 _hack_acc=False, p3b_pool_cols=1248, merged=False, mp_pool_cols=2048,
                 dp_first=False, consts_on_gpsimd=True, slim_tail=False,
                 p2_pool_cols=2048):
    nc = bass.Bass("TRN2", target_bir_lowering=False, debug=False,
                   num_devices=N_CORES)

    d_ext = nc.dram_tensor("D", [B_SHARD, C], F32R, kind="ExternalInput").ap()
    dp_ext = nc.dram_tensor("Dp", [B_SHARD, C], F32R, kind="ExternalInput").ap()
    ident_ext = nc.dram_tensor("ident", [128, 128], F32R, kind="ExternalInput").ap()
    whalf_ext = nc.dram_tensor("whalf", [128, 1], F32R, kind="ExternalInput").ap()
    wneg_ext = nc.dram_tensor("wneg", [128, 1], F32R, kind="ExternalInput").ap()
    out_ext = nc.dram_tensor("colsum", [1, C], F32, kind="ExternalOutput").ap()

    tc_cls = _SlimTailTileContext if slim_tail else tile.TileContext
    with tc_cls(nc) as tc:
        with tc.tile_pool(name="consts", bufs=1) as consts, \
             tc.tile_pool(name="inp", bufs=inp_bufs) as inp, \
             tc.tile_pool(name="logs", bufs=logs_bufs) as logs, \
             tc.tile_pool(name="prods", bufs=prods_bufs) as prods, \
             tc.tile_pool(name="half", bufs=halfp_bufs) as halfp, \
             tc.tile_pool(name="ps_s", bufs=ps_s_bufs, space="PSUM") as ps_s, \
             tc.tile_pool(name="ps_acc", bufs=1, space="PSUM") as ps_acc:

            c_eng = nc.gpsimd if consts_on_gpsimd else nc.sync
            ident = consts.tile([128, 128], F32R)
            c_eng.dma_start(ident[:], ident_ext[:])
            whalf = consts.tile([128, 1], F32R)
            c_eng.dma_start(whalf[:], whalf_ext[:])
            wneg = consts.tile([128, 1], F32R)
            c_eng.dma_start(wneg[:], wneg_ext[:])
            bias_eps = consts.tile([128, 1], F32)
            nc.vector.memset(bias_eps[:], EPS / 2)

            if _hack_acc:
                acc = ps_acc.tile([1, 512], F32)
            elif acc_onebank:
                acc = ps_acc.tile([128, 512], F32)
            else:
                acc = ps_acc.tile([1, C], F32)

            def emit_reds(p1, p2, p3s, first, last):
                if isinstance(p3s, tuple) and p3s and p3s[0] == "four":
                    _, p3a, p3b = p3s
                    for cc in range(4):
                        cs = slice(cc * 512, (cc + 1) * 512)
                        nc.tensor.matmul(acc[:, cs], whalf[:], p1[:, cs],
                                         start=first, stop=False)
                        nc.tensor.matmul(acc[:, cs], whalf[:], p2[:, cs],
                                         start=False, stop=False)
                        nc.tensor.matmul(acc[:, cs], wneg[:], p3a[:, cs],
                                         start=False, stop=False)
                        nc.tensor.matmul(acc[:, cs], wneg[:], p3b[:, cs],
                                         start=False, stop=last)
                    return
                for h in range(2):
                    p3 = p3s[h]
                    for q in range(2):
                        cc = 2 * h + q
                        cs = slice(h * 1024 + q * 512, h * 1024 + (q + 1) * 512)
                        ps = slice(q * 512, (q + 1) * 512)
                        if _hack_acc:
                            acc_out = acc[:, :]
                            tp = None
                        elif acc_onebank:
                            acc_out = acc[32 * cc:32 * cc + 1, :]
                            tp = (0, 32 * cc)
                        else:
                            acc_out = acc[:, cs]
                            tp = None
                        nc.tensor.matmul(acc_out, whalf[:], p1[:, cs],
                                         start=first, stop=False,
                                         tile_position=tp)
                        nc.tensor.matmul(acc_out, whalf[:], p2[:, cs],
                                         start=False, stop=False,
                                         tile_position=tp)
                        nc.tensor.matmul(acc_out, wneg[:], p3[:, ps],
                                         start=False, stop=last,
                                         tile_position=tp)

            pending = None
            for rt in range(ROW_TILES):
                rows = slice(rt * 128, (rt + 1) * 128)
                if merged:
                    # One [128, 2C] tile: D in cols [0, C), D_pred in [C, 2C).
                    # One Ln covers both logs; one product op pair splits at
                    # an arbitrary column for DVE/Pool balance.
                    ddp = inp.tile([128, 2 * C], F32R, tag="ddp")
                    cw = C // dma_split
                    for k in range(dma_split):
                        sl = slice(k * cw, (k + 1) * cw)
                        sl2 = slice(C + k * cw, C + (k + 1) * cw)
                        nc.sync.dma_start(ddp[:, sl], d_ext[rows, sl])
                        nc.sync.dma_start(ddp[:, sl2], dp_ext[rows, sl])
                    log_ddp = logs.tile([128, 2 * C], F32, tag="logddp")
                    nc.scalar.activation(log_ddp[:], ddp[:].bitcast(F32), LN)
                    pm = prods.tile([128, 2 * C], F32R, tag="pm")
                    mx = 2 * C - mp_pool_cols
                    if mx > 0:
                        nc.vector.tensor_tensor(pm[:, :mx],
                                                ddp[:, :mx].bitcast(F32),
                                                log_ddp[:, :mx], MULT)
                    if mx < 2 * C:
                        nc.gpsimd.tensor_tensor(pm[:, mx:],
                                                ddp[:, mx:].bitcast(F32),
                                                log_ddp[:, mx:], MULT)
                    d_t = ddp[:, :C]
                    dp_t = ddp[:, C:]
                    p1 = pm[:, :C]
                    p2 = pm[:, C:]
                else:
                    d_t = inp.tile([128, C], F32R, tag="d")
                    dp_t = inp.tile([128, C], F32R, tag="dp",
                                    bufs=dp_bufs or inp_bufs)
                    cw = C // dma_split
                    eng_d = getattr(nc, {"sync": "sync", "scalar": "scalar",
                                         "vector": "vector"}[dma_engs[0]])
                    eng_dp = getattr(nc, {"sync": "sync", "scalar": "scalar",
                                          "vector": "vector"}[dma_engs[1]])
                    for k in range(dma_split):
                        sl = slice(k * cw, (k + 1) * cw)
                        eng_d.dma_start(d_t[:, sl], d_ext[rows, sl])
                        eng_dp.dma_start(dp_t[:, sl], dp_ext[rows, sl])

                    p2_eng = nc.gpsimd if p2_engine == "gpsimd" else nc.vector
                    if dp_first:
                        # logDp first so the slow Pool product starts ~1.9us
                        # earlier; p1 is emitted after the h-loop so DVE's
                        # in-order stream never stalls waiting for logD.
                        log_dp = logs.tile([128, C], F32, tag="logdp")
                        nc.scalar.activation(log_dp[:], dp_t[:].bitcast(F32),
                                             LN)
                        p2 = prods.tile([128, C], F32R, tag="p2")
                        pc = p2_pool_cols if p2_engine == "gpsimd" else 0
                        if pc > 0:
                            nc.gpsimd.tensor_tensor(p2[:, :pc],
                                                    dp_t[:, :pc].bitcast(F32),
                                                    log_dp[:, :pc], MULT)
                        if pc < C:
                            nc.vector.tensor_tensor(p2[:, pc:],
                                                    dp_t[:, pc:].bitcast(F32),
                                                    log_dp[:, pc:], MULT)
                        log_d = logs.tile([128, C], F32, tag="logd")
                        nc.scalar.activation(log_d[:], d_t[:].bitcast(F32), LN)
                        p1 = None
                    else:
                        log_d = logs.tile([128, C], F32, tag="logd")
                        nc.scalar.activation(log_d[:], d_t[:].bitcast(F32), LN)
                        log_dp = logs.tile([128, C], F32, tag="logdp")
                        nc.scalar.activation(log_dp[:], dp_t[:].bitcast(F32),
                                             LN)

                        p1 = prods.tile([128, C], F32R, tag="p1")
                        nc.vector.tensor_tensor(p1[:], d_t[:].bitcast(F32),
                                                log_d[:], MULT)
                        p2 = prods.tile([128, C], F32R, tag="p2")
                        pc = p2_pool_cols if p2_engine == "gpsimd" else 0
                        if pc > 0:
                            nc.gpsimd.tensor_tensor(p2[:, :pc],
                                                    dp_t[:, :pc].bitcast(F32),
                                                    log_dp[:, :pc], MULT)
                        if pc < C:
                            nc.vector.tensor_tensor(p2[:, pc:],
                                                    dp_t[:, pc:].bitcast(F32),
                                                    log_dp[:, pc:], MULT)

                if s_mode == "four_prod":
                    logm = halfp.tile([128, C], F32, tag="logm")
                    for h in range(2):
                        s_ps = ps_s.tile([128, 1024], F32, tag="s")
                        for q in range(2):
                            cs = slice(h * 1024 + q * 512,
                                       h * 1024 + (q + 1) * 512)
                            ps = slice(q * 512, (q + 1) * 512)
                            nc.tensor.matmul(s_ps[:, ps], ident[:], d_t[:, cs],
                                             start=True, stop=False)
                            nc.tensor.matmul(s_ps[:, ps], ident[:],
                                             dp_t[:, cs], start=False,
                                             stop=True)
                        nc.scalar.activation(logm[:, h * 1024:(h + 1) * 1024],
                                             s_ps[:], LN,
                                             scale=0.5, bias=bias_eps[:])
                    p3a = prods.tile([128, C], F32R, tag="p3a")
                    nc.vector.tensor_tensor(p3a[:], d_t[:].bitcast(F32),
                                            logm[:], MULT)
                    p3b = prods.tile([128, C], F32R, tag="p3b")
                    pc = p3b_pool_cols
                    if pc > 0:
                        nc.gpsimd.tensor_tensor(p3b[:, :pc],
                                                dp_t[:, :pc].bitcast(F32),
                                                logm[:, :pc], MULT)
                    if pc < C:
                        nc.vector.tensor_tensor(p3b[:, pc:],
                                                dp_t[:, pc:].bitcast(F32),
                                                logm[:, pc:], MULT)
                    p3s = ("four", p3a, p3b)
                elif s_mode == "split_sbuf":
                    s_t = halfp.tile([128, C], F32, tag="s")
                    pc = s_pool_cols
                    if pc > 0:
                        nc.gpsimd.tensor_tensor(
                            s_t[:, :pc], d_t[:, :pc].bitcast(F32),
                            dp_t[:, :pc].bitcast(F32), ADD)
                    if pc < C:
                        nc.vector.tensor_tensor(
                            s_t[:, pc:], d_t[:, pc:].bitcast(F32),
                            dp_t[:, pc:].bitcast(F32), ADD)
                    logm = halfp.tile([128, C], F32, tag="logm")
                    nc.scalar.activation(logm[:], s_t[:], LN,
                                         scale=0.5, bias=bias_eps[:])
                    p3 = halfp.tile([128, C], F32R, tag="p3")
                    nc.vector.tensor_tensor(p3[:], s_t[:], logm[:], MULT)
                    p3s = [p3[:, :1024], p3[:, 1024:]]
                else:
                    p3s = []
                    for h in range(2):
                        s_ps = ps_s.tile([128, 1024], F32, tag="s")
                        for q in range(2):
                            cs = slice(h * 1024 + q * 512,
                                       h * 1024 + (q + 1) * 512)
                            ps = slice(q * 512, (q + 1) * 512)
                            nc.tensor.matmul(s_ps[:, ps], ident[:], d_t[:, cs],
                                             start=True, stop=False)
                            nc.tensor.matmul(s_ps[:, ps], ident[:],
                                             dp_t[:, cs], start=False,
                                             stop=True)

                        logm = halfp.tile([128, 1024], F32, tag="logm")
                        nc.scalar.activation(logm[:], s_ps[:], LN,
                                             scale=0.5, bias=bias_eps[:])
                        p3 = halfp.tile([128, 1024], F32R, tag="p3")
                        nc.vector.tensor_tensor(p3[:], s_ps[:], logm[:], MULT)
                        p3s.append(p3)

                if not merged and dp_first and p1 is None:
                    p1 = prods.tile([128, C], F32R, tag="p1")
                    nc.vector.tensor_tensor(p1[:], d_t[:].bitcast(F32),
                                            log_d[:], MULT)

                if not pipelined_reds:
                    emit_reds(p1, p2, p3s, rt == 0, rt == ROW_TILES - 1)
                else:
                    if pending is not None:
                        emit_reds(pending[0], pending[1], pending[2],
                                  pending[3], False)
                    pending = (p1, p2, p3s, rt == 0)
            if pipelined_reds and pending is not None:
                emit_reds(pending[0], pending[1], pending[2],
                          pending[3], True)

            if acc_onebank:
                out_sb = consts.tile([128, 512], F32)
                for cc in range(4):
                    pr = slice(32 * cc, 32 * cc + 1)
                    nc.vector.tensor_copy(out_sb[pr, :], acc[pr, :])
                    nc.sync.dma_start(
                        out_ext[0:1, cc * 512:(cc + 1) * 512], out_sb[pr, :])
            else:
                out_sb = consts.tile([1, C], F32)
                for q in range(4):
                    ps = slice(q * 512, (q + 1) * 512)
                    src_ps = slice(0, 512) if _hack_acc else ps
                    # split the single-partition PSUM->SBUF copies across
                    # DVE and ACT so they drain in parallel at kernel tail
                    eng = nc.vector if q % 2 == 0 else nc.scalar
                    if eng is nc.vector:
                        eng.tensor_copy(out_sb[:, ps], acc[:, src_ps])
                    else:
                        nc.scalar.copy(out_sb[:, ps], acc[:, src_ps])
                nc.sync.dma_start(out_ext[:], out_sb[:])

    _fix_multi_waits(nc)
    return nc


_NC_CACHE = None


def _get_nc():
    global _NC_CACHE
    if _NC_CACHE is None:
        _NC_CACHE = build_kernel()
    return _NC_CACHE


def make_in_maps(D, D_pred):
    D = np.ascontiguousarray(np.asarray(D, dtype=np.float32))
    D_pred = np.ascontiguousarray(np.asarray(D_pred, dtype=np.float32))
    ident = np.eye(128, dtype=np.float32)
    whalf = np.full((128, 1), 0.5, dtype=np.float32)
    wneg = np.full((128, 1), -0.5, dtype=np.float32)
    in_maps = []
    for c in range(N_CORES):
        rows = slice(c * B_SHARD, (c + 1) * B_SHARD)
        in_maps.append({
            "D": D[rows],
            "Dp": D_pred[rows],
            "ident": ident,
            "whalf": whalf,
            "wneg": wneg,
        })
    return in_maps


def finish(results):
    colsum = np.zeros(C, dtype=np.float64)
    for c in range(N_CORES):
        colsum += results[c]["colsum"][0].astype(np.float64)
    w = (C - np.arange(C)).astype(np.float64)
    return np.asarray((colsum * w).sum() / B, dtype=np.float32)


def kernel(D, D_pred):
    nc = _get_nc()
    res = run_bass_kernel_spmd(nc, make_in_maps(D, D_pred), list(range(N_CORES)))
    return finish(res.results)


if __name__ == "__main__":
    rng = np.random.default_rng(0)
    D = rng.uniform(1e-4, 1.0, (B, C)).astype(np.float32)
    Dp = rng.uniform(1e-4, 1.0, (B, C)).astype(np.float32)
    out = kernel(D, Dp)
    print("kernel out:", out)


# revision 28
# speedup vs baseline: 1.0909x; 1.0755x over previous
"""CJS loss kernel for Trainium2 (8 NeuronCores, SPMD data-parallel).

loss = sum_{b,j} w_j * e[b,j] / B     with w_j = C - j and
e = 0.5*(D*(logD - logm) + Dp*(logDp - logm)),  m = 0.5*(D + Dp + EPS)

Sharding: batch dim B=16384 split into 8 shards of 2048 rows; each core
reduces its shard to per-column sums colsum[j] = sum_b e[b,j] (output
[1, 2048] per core); the host finishes with the tiny weighted dot and
the divide by B.

Per 128-row tile on each core (default config):
  ACT :  logD = Ln(D), logDp = Ln(Dp), logm = Ln(0.5*s + EPS/2) from PSUM
  PE  :  s = identity@D + identity@Dp   (f32r matmuls into PSUM)
         colsum += 0.5*ones@p1 + 0.5*ones@p2 - 0.5*ones@p3  (PSUM
         accumulation across all 16 row tiles; the reduction matmuls for
         tile k are emitted in iteration k+1 so the next tile's identity
         matmuls always lead on the in-order PE queue)
  DVE :  p1 = D*logD, p3 = s*logm
  POOL:  p2 = Dp*logDp   (GpSimd, ~2x slower/elem but runs in parallel)

Every engine lands under the per-core DMA roofline (33.5 MB / 358 GB/s
= 93.5 us): cost-model busy times are ACT 93.8, DMA 93.5, DVE 75.9,
Pool 66.9, PE 69.2 us; predicted wall 118 us (pipeline ramp + Tile
drain/barrier tail account for the rest).

float32r (TF32-like single-pass PE mode, 1 cycle/row vs 4 for fp32) is
used for every matmul operand. The BIR verifier demands f32r-rounded
producers, so the DRAM params and input tiles are declared f32r (numpy
f32 bits feed them directly; PE rounds internally) and the DVE/Pool
product tiles are written as f32r. Measured end-to-end relative error
on hardware: 9.3e-6.

Non-default build_kernel() options (split_sbuf / four_prod / acc_onebank
variants) are kept for reference; they all simmed slower.
"""

import sys

sys.path.insert(0, "/opt/trn_rl_repo")

import numpy as np

import concourse.bass as bass
import concourse.mybir as mybir
from concourse import tile
from concourse.bass_utils import run_bass_kernel_spmd

N_CORES = 8
B, C = 16384, 2048
B_SHARD = B // N_CORES          # 2048 rows per core
ROW_TILES = B_SHARD // 128      # 16 tiles of 128 rows
EPS = 1e-8

F32 = mybir.dt.float32
F32R = mybir.dt.float32r
LN = mybir.ActivationFunctionType.Ln
MULT = mybir.AluOpType.mult
ADD = mybir.AluOpType.add


def _fix_multi_waits(nc):
    """This walrus build accepts only one sync-wait per instruction, while
    Tile's sem-assigner can attach several. Split extras onto single-wait
    EventSemaphore carriers inserted before the instruction (same engine,
    identical blocking semantics)."""
    for fn in nc.m.functions:
        for blk in fn.blocks:
            insns = list(blk.instructions)
            out = []
            changed = False
            for ins in insns:
                si = ins.sync_info
                waits = list(si.on_wait) if si is not None else []
                if len(waits) > 1:
                    for k, w in enumerate(waits[:-1]):
                        carrier = mybir.InstEventSemaphore(
                            name=f"{ins.name}-wsplit{k}",
                            engine=ins.engine,
                            ins=[],
                            outs=[],
                            sync_info=mybir.SyncInfo(on_wait=[w], on_update=[]),
                        )
                        carrier.debug = ins.debug
                        nc.register_instruction(carrier)
                        out.append(carrier)
                    ins.sync_info = mybir.SyncInfo(
                        on_wait=[waits[-1]], on_update=list(si.on_update)
                    )
                    changed = True
                out.append(ins)
            if changed:
                blk.instructions.clear()
                blk.instructions.extend(out)


class _SlimTailTileContext(tile.TileContext):
    """Skip the second all-engine barrier of the stock kernel tail (the one
    after clear_and_free_semaphores). The final drain already waited for all
    outstanding work and the first barrier aligned the engines; the second
    only guards sem-clears against an engine racing ahead into... nothing
    (end of program). Validated by repeat executions on hardware."""

    def _drain_and_barrier(self, tick_clock, wait_clock):
        from concourse.vector_clock import ScopedClock
        drain_inst = self.nc.sync.drain()
        wait_clock.add_sem_waits(
            drain_inst.ins, ScopedClock({None: tick_clock.global_clock})
        )
        self.nc.all_engine_barrier()
        assert self.sems is not None
        popped = self.nc._tile_sem_poison_stack.pop()
        assert popped is self._sem_poison
        self.nc.clear_and_free_semaphores(list(self.sems.allocated().values()))


def build_kernel(p2_engine="gpsimd", inp_bufs=3, dp_bufs=None, logs_bufs=2, prods_bufs=3, halfp_bufs=4, ps_s_bufs=2, dma_split=4, pipelined_reds=True, acc_onebank=False,
                 s_mode="pe_psum", s_pool_cols=1024, dma_engs=("sync", "sync"),
                 _hack_acc=False, p3b_pool_cols=1248, merged=False, mp_pool_cols=2048,
                 dp_first=False, consts_on_gpsimd=True, slim_tail=False,
                 p2_pool_cols=832, p1_pool_cols=512, ln_preload=False):
    nc = bass.Bass("TRN2", target_bir_lowering=False, debug=False,
                   num_devices=N_CORES)

    d_ext = nc.dram_tensor("D", [B_SHARD, C], F32R, kind="ExternalInput").ap()
    dp_ext = nc.dram_tensor("Dp", [B_SHARD, C], F32R, kind="ExternalInput").ap()
    ident_ext = nc.dram_tensor("ident", [128, 128], F32R, kind="ExternalInput").ap()
    whalf_ext = nc.dram_tensor("whalf", [128, 1], F32R, kind="ExternalInput").ap()
    wneg_ext = nc.dram_tensor("wneg", [128, 1], F32R, kind="ExternalInput").ap()
    out_ext = nc.dram_tensor("colsum", [1, C], F32, kind="ExternalOutput").ap()

    tc_cls = _SlimTailTileContext if slim_tail else tile.TileContext
    with tc_cls(nc) as tc:
        with tc.tile_pool(name="consts", bufs=1) as consts, \
             tc.tile_pool(name="inp", bufs=inp_bufs) as inp, \
             tc.tile_pool(name="logs", bufs=logs_bufs) as logs, \
             tc.tile_pool(name="prods", bufs=prods_bufs) as prods, \
             tc.tile_pool(name="half", bufs=halfp_bufs) as halfp, \
             tc.tile_pool(name="ps_s", bufs=ps_s_bufs, space="PSUM") as ps_s, \
             tc.tile_pool(name="ps_acc", bufs=1, space="PSUM") as ps_acc:

            c_eng = nc.gpsimd if consts_on_gpsimd else nc.sync
            ident = consts.tile([128, 128], F32R)
            c_eng.dma_start(ident[:], ident_ext[:])
            whalf = consts.tile([128, 1], F32R)
            c_eng.dma_start(whalf[:], whalf_ext[:])
            wneg = consts.tile([128, 1], F32R)
            c_eng.dma_start(wneg[:], wneg_ext[:])
            bias_eps = consts.tile([128, 1], F32)
            nc.vector.memset(bias_eps[:], EPS / 2)
            if ln_preload:
                # tiny Ln so the ACT table set loads while the first input
                # DMAs stream, instead of stalling the first real log
                warm = consts.tile([128, 1], F32)
                nc.scalar.activation(warm[:], bias_eps[:], LN)

            if _hack_acc:
                acc = ps_acc.tile([1, 512], F32)
            elif acc_onebank:
                acc = ps_acc.tile([128, 512], F32)
            else:
                acc = ps_acc.tile([1, C], F32)

            def emit_reds(p1, p2, p3s, first, last):
                if isinstance(p3s, tuple) and p3s and p3s[0] == "four":
                    _, p3a, p3b = p3s
                    for cc in range(4):
                        cs = slice(cc * 512, (cc + 1) * 512)
                        nc.tensor.matmul(acc[:, cs], whalf[:], p1[:, cs],
                                         start=first, stop=False)
                        nc.tensor.matmul(acc[:, cs], whalf[:], p2[:, cs],
                                         start=False, stop=False)
                        nc.tensor.matmul(acc[:, cs], wneg[:], p3a[:, cs],
                                         start=False, stop=False)
                        nc.tensor.matmul(acc[:, cs], wneg[:], p3b[:, cs],
                                         start=False, stop=last)
                    return
                for h in range(2):
                    p3 = p3s[h]
                    for q in range(2):
                        cc = 2 * h + q
                        cs = slice(h * 1024 + q * 512, h * 1024 + (q + 1) * 512)
                        ps = slice(q * 512, (q + 1) * 512)
                        if _hack_acc:
                            acc_out = acc[:, :]
                            tp = None
                        elif acc_onebank:
                            acc_out = acc[32 * cc:32 * cc + 1, :]
                            tp = (0, 32 * cc)
                        else:
                            acc_out = acc[:, cs]
                            tp = None
                        nc.tensor.matmul(acc_out, whalf[:], p1[:, cs],
                                         start=first, stop=False,
                                         tile_position=tp)
                        nc.tensor.matmul(acc_out, whalf[:], p2[:, cs],
                                         start=False, stop=False,
                                         tile_position=tp)
                        nc.tensor.matmul(acc_out, wneg[:], p3[:, ps],
                                         start=False, stop=last,
                                         tile_position=tp)

            pending = None
            for rt in range(ROW_TILES):
                rows = slice(rt * 128, (rt + 1) * 128)
                if merged:
                    # One [128, 2C] tile: D in cols [0, C), D_pred in [C, 2C).
                    # One Ln covers both logs; one product op pair splits at
                    # an arbitrary column for DVE/Pool balance.
                    ddp = inp.tile([128, 2 * C], F32R, tag="ddp")
                    cw = C // dma_split
                    for k in range(dma_split):
                        sl = slice(k * cw, (k + 1) * cw)
                        sl2 = slice(C + k * cw, C + (k + 1) * cw)
                        nc.sync.dma_start(ddp[:, sl], d_ext[rows, sl])
                        nc.sync.dma_start(ddp[:, sl2], dp_ext[rows, sl])
                    log_ddp = logs.tile([128, 2 * C], F32, tag="logddp")
                    nc.scalar.activation(log_ddp[:], ddp[:].bitcast(F32), LN)
                    pm = prods.tile([128, 2 * C], F32R, tag="pm")
                    mx = 2 * C - mp_pool_cols
                    if mx > 0:
                        nc.vector.tensor_tensor(pm[:, :mx],
                                                ddp[:, :mx].bitcast(F32),
                                                log_ddp[:, :mx], MULT)
                    if mx < 2 * C:
                        nc.gpsimd.tensor_tensor(pm[:, mx:],
                                                ddp[:, mx:].bitcast(F32),
                                                log_ddp[:, mx:], MULT)
                    d_t = ddp[:, :C]
                    dp_t = ddp[:, C:]
                    p1 = pm[:, :C]
                    p2 = pm[:, C:]
                else:
                    d_t = inp.tile([128, C], F32R, tag="d")
                    dp_t = inp.tile([128, C], F32R, tag="dp",
                                    bufs=dp_bufs or inp_bufs)
                    cw = C // dma_split
                    eng_d = getattr(nc, {"sync": "sync", "scalar": "scalar",
                                         "vector": "vector"}[dma_engs[0]])
                    eng_dp = getattr(nc, {"sync": "sync", "scalar": "scalar",
                                          "vector": "vector"}[dma_engs[1]])
                    for k in range(dma_split):
                        sl = slice(k * cw, (k + 1) * cw)
                        eng_d.dma_start(d_t[:, sl], d_ext[rows, sl])
                        eng_dp.dma_start(dp_t[:, sl], dp_ext[rows, sl])

                    p2_eng = nc.gpsimd if p2_engine == "gpsimd" else nc.vector
                    if dp_first:
                        # logDp first so the slow Pool product starts ~1.9us
                        # earlier; p1 is emitted after the h-loop so DVE's
                        # in-order stream never stalls waiting for logD.
                        log_dp = logs.tile([128, C], F32, tag="logdp")
                        nc.scalar.activation(log_dp[:], dp_t[:].bitcast(F32),
                                             LN)
                        p2 = prods.tile([128, C], F32R, tag="p2")
                        pc = p2_pool_cols if p2_engine == "gpsimd" else 0
                        if pc > 0:
                            nc.gpsimd.tensor_tensor(p2[:, :pc],
                                                    dp_t[:, :pc].bitcast(F32),
                                                    log_dp[:, :pc], MULT)
                        if pc < C:
                            nc.vector.tensor_tensor(p2[:, pc:],
                                                    dp_t[:, pc:].bitcast(F32),
                                                    log_dp[:, pc:], MULT)
                        log_d = logs.tile([128, C], F32, tag="logd")
                        nc.scalar.activation(log_d[:], d_t[:].bitcast(F32), LN)
                        p1 = None
                    else:
                        log_d = logs.tile([128, C], F32, tag="logd")
                        nc.scalar.activation(log_d[:], d_t[:].bitcast(F32), LN)
                        log_dp = logs.tile([128, C], F32, tag="logdp")
                        nc.scalar.activation(log_dp[:], dp_t[:].bitcast(F32),
                                             LN)

                        p1 = prods.tile([128, C], F32R, tag="p1")
                        pc1 = p1_pool_cols if p2_engine == "gpsimd" else 0
                        if pc1 > 0:
                            nc.gpsimd.tensor_tensor(p1[:, :pc1],
                                                    d_t[:, :pc1].bitcast(F32),
                                                    log_d[:, :pc1], MULT)
                        if pc1 < C:
                            nc.vector.tensor_tensor(p1[:, pc1:],
                                                    d_t[:, pc1:].bitcast(F32),
                                                    log_d[:, pc1:], MULT)
                        p2 = prods.tile([128, C], F32R, tag="p2")
                        pc = p2_pool_cols if p2_engine == "gpsimd" else 0
                        if pc > 0:
                            nc.gpsimd.tensor_tensor(p2[:, :pc],
                                                    dp_t[:, :pc].bitcast(F32),
                                                    log_dp[:, :pc], MULT)
                        if pc < C:
                            nc.vector.tensor_tensor(p2[:, pc:],
                                                    dp_t[:, pc:].bitcast(F32),
                                                    log_dp[:, pc:], MULT)

                if s_mode == "four_prod":
                    logm = halfp.tile([128, C], F32, tag="logm")
                    for h in range(2):
                        s_ps = ps_s.tile([128, 1024], F32, tag="s")
                        for q in range(2):
                            cs = slice(h * 1024 + q * 512,
                                       h * 1024 + (q + 1) * 512)
                            ps = slice(q * 512, (q + 1) * 512)
                            nc.tensor.matmul(s_ps[:, ps], ident[:], d_t[:, cs],
                                             start=True, stop=False)
                            nc.tensor.matmul(s_ps[:, ps], ident[:],
                                             dp_t[:, cs], start=False,
                                             stop=True)
                        nc.scalar.activation(logm[:, h * 1024:(h + 1) * 1024],
                                             s_ps[:], LN,
                                             scale=0.5, bias=bias_eps[:])
                    p3a = prods.tile([128, C], F32R, tag="p3a")
                    nc.vector.tensor_tensor(p3a[:], d_t[:].bitcast(F32),
                                            logm[:], MULT)
                    p3b = prods.tile([128, C], F32R, tag="p3b")
                    pc = p3b_pool_cols
                    if pc > 0:
                        nc.gpsimd.tensor_tensor(p3b[:, :pc],
                                                dp_t[:, :pc].bitcast(F32),
                                                logm[:, :pc], MULT)
                    if pc < C:
                        nc.vector.tensor_tensor(p3b[:, pc:],
                                                dp_t[:, pc:].bitcast(F32),
                                                logm[:, pc:], MULT)
                    p3s = ("four", p3a, p3b)
                elif s_mode == "split_sbuf":
                    s_t = halfp.tile([128, C], F32, tag="s")
                    pc = s_pool_cols
                    if pc > 0:
                        nc.gpsimd.tensor_tensor(
                            s_t[:, :pc], d_t[:, :pc].bitcast(F32),
                            dp_t[:, :pc].bitcast(F32), ADD)
                    if pc < C:
                        nc.vector.tensor_tensor(
                            s_t[:, pc:], d_t[:, pc:].bitcast(F32),
                            dp_t[:, pc:].bitcast(F32), ADD)
                    logm = halfp.tile([128, C], F32, tag="logm")
                    nc.scalar.activation(logm[:], s_t[:], LN,
                                         scale=0.5, bias=bias_eps[:])
                    p3 = halfp.tile([128, C], F32R, tag="p3")
                    nc.vector.tensor_tensor(p3[:], s_t[:], logm[:], MULT)
                    p3s = [p3[:, :1024], p3[:, 1024:]]
                else:
                    p3s = []
                    for h in range(2):
                        s_ps = ps_s.tile([128, 1024], F32, tag="s")
                        for q in range(2):
                            cs = slice(h * 1024 + q * 512,
                                       h * 1024 + (q + 1) * 512)
                            ps = slice(q * 512, (q + 1) * 512)
                            nc.tensor.matmul(s_ps[:, ps], ident[:], d_t[:, cs],
                                             start=True, stop=False)
                            nc.tensor.matmul(s_ps[:, ps], ident[:],
                                             dp_t[:, cs], start=False,
                                             stop=True)

                        logm = halfp.tile([128, 1024], F32, tag="logm")
                        nc.scalar.activation(logm[:], s_ps[:], LN,
                                             scale=0.5, bias=bias_eps[:])
                        p3 = halfp.tile([128, 1024], F32R, tag="p3")
                        nc.vector.tensor_tensor(p3[:], s_ps[:], logm[:], MULT)
                        p3s.append(p3)

                if not merged and dp_first and p1 is None:
                    p1 = prods.tile([128, C], F32R, tag="p1")
                    nc.vector.tensor_tensor(p1[:], d_t[:].bitcast(F32),
                                            log_d[:], MULT)

                if not pipelined_reds:
                    emit_reds(p1, p2, p3s, rt == 0, rt == ROW_TILES - 1)
                else:
                    if pending is not None:
                        emit_reds(pending[0], pending[1], pending[2],
                                  pending[3], False)
                    pending = (p1, p2, p3s, rt == 0)
            if pipelined_reds and pending is not None:
                emit_reds(pending[0], pending[1], pending[2],
                          pending[3], True)

            if acc_onebank:
                out_sb = consts.tile([128, 512], F32)
                for cc in range(4):
                    pr = slice(32 * cc, 32 * cc + 1)
                    nc.vector.tensor_copy(out_sb[pr, :], acc[pr, :])
                    nc.sync.dma_start(
                        out_ext[0:1, cc * 512:(cc + 1) * 512], out_sb[pr, :])
            else:
                out_sb = consts.tile([1, C], F32)
                for q in range(4):
                    ps = slice(q * 512, (q + 1) * 512)
                    src_ps = slice(0, 512) if _hack_acc else ps
                    # split the single-partition PSUM->SBUF copies across
                    # DVE and ACT so they drain in parallel at kernel tail
                    eng = nc.vector if q % 2 == 0 else nc.scalar
                    if eng is nc.vector:
                        eng.tensor_copy(out_sb[:, ps], acc[:, src_ps])
                    else:
                        nc.scalar.copy(out_sb[:, ps], acc[:, src_ps])
                nc.sync.dma_start(out_ext[:], out_sb[:])

    _fix_multi_waits(nc)
    return nc


_NC_CACHE = None


def _get_nc():
    global _NC_CACHE
    if _NC_CACHE is None:
        _NC_CACHE = build_kernel()
    return _NC_CACHE


def make_in_maps(D, D_pred):
    D = np.ascontiguousarray(np.asarray(D, dtype=np.float32))
    D_pred = np.ascontiguousarray(np.asarray(D_pred, dtype=np.float32))
    ident = np.eye(128, dtype=np.float32)
    whalf = np.full((128, 1), 0.5, dtype=np.float32)
    wneg = np.full((128, 1), -0.5, dtype=np.float32)
    in_maps = []
    for c in range(N_CORES):
        rows = slice(c * B_SHARD, (c + 1) * B_SHARD)
        in_maps.append({
            "D": D[rows],
            "Dp": D_pred[rows],
            "ident": ident,
            "whalf": whalf,
            "wneg": wneg,
        })
    return in_maps


def finish(results):
    colsum = np.zeros(C, dtype=np.float64)
    for c in range(N_CORES):
        colsum += results[c]["colsum"][0].astype(np.float64)
    w = (C - np.arange(C)).astype(np.float64)
    return np.asarray((colsum * w).sum() / B, dtype=np.float32)


def kernel(D, D_pred):
    nc = _get_nc()
    res = run_bass_kernel_spmd(nc, make_in_maps(D, D_pred), list(range(N_CORES)))
    return finish(res.results)


if __name__ == "__main__":
    rng = np.random.default_rng(0)
    D = rng.uniform(1e-4, 1.0, (B, C)).astype(np.float32)
    Dp = rng.uniform(1e-4, 1.0, (B, C)).astype(np.float32)
    out = kernel(D, Dp)
    print("kernel out:", out)


# revision 32
# speedup vs baseline: 1.0935x; 1.0024x over previous
"""CJS loss kernel for Trainium2 (8 NeuronCores, SPMD data-parallel).

loss = sum_{b,j} w_j * e[b,j] / B     with w_j = C - j and
e = 0.5*(D*(logD - logm) + Dp*(logDp - logm)),  m = 0.5*(D + Dp + EPS)

Sharding: batch dim B=16384 split into 8 shards of 2048 rows; each core
reduces its shard to per-column sums colsum[j] = sum_b e[b,j] (output
[1, 2048] per core); the host finishes with the tiny weighted dot and
the divide by B.

Per 128-row tile on each core (default config):
  ACT :  logD = Ln(D), logDp = Ln(Dp), logm = Ln(0.5*s + EPS/2) from PSUM
  PE  :  s = identity@D + identity@Dp   (f32r matmuls into PSUM)
         colsum += 0.5*ones@p1 + 0.5*ones@p2 - 0.5*ones@p3  (PSUM
         accumulation across all 16 row tiles; the reduction matmuls for
         tile k are emitted in iteration k+1 so the next tile's identity
         matmuls always lead on the in-order PE queue)
  DVE :  p3 = s*logm, plus the right 1536 cols of p1 and 1216 of p2
  POOL:  left 512 cols of p1 + left 832 of p2 (GpSimd is ~2x slower per
         element, but short Pool ops keep it off every dependency chain)

The kernel is ACT-serial-bound: total = ACT ramp (~7.5 us, first-tile
DMA) + ACT busy (95.0 us; the 3 mandatory logs at 1 elem/lane/cycle)
+ unwind tail (~5.9 us). Cost-model busy: ACT 95.0, DMA 93.5 (the
33.5 MB / 358 GB/s roofline), DVE 84.1, PE 72.0, Pool 55.3 us;
predicted wall 108.3 us = 1.16x the DMA roofline. Tuned by simulator
sweep: dma_split=4, product column-splits as above, const DMAs issued
from the GpSimd queue, tail PSUM->SBUF copies split across DVE/ACT,
reduction matmuls pipelined one iteration.

float32r (TF32-like single-pass PE mode, 1 cycle/row vs 4 for fp32) is
used for every matmul operand. The BIR verifier demands f32r-rounded
producers, so the DRAM params and input tiles are declared f32r (numpy
f32 bits feed them directly; PE rounds internally) and the DVE/Pool
product tiles are written as f32r. Measured end-to-end relative error
on hardware: 9.25e-6 (all 8 cores, repeat executions verified).

Non-default build_kernel() options (split_sbuf / four_prod / acc_onebank
variants) are kept for reference; they all simmed slower.
"""

import sys

sys.path.insert(0, "/opt/trn_rl_repo")

import numpy as np

import concourse.bass as bass
import concourse.mybir as mybir
from concourse import tile
from concourse.bass_utils import run_bass_kernel_spmd

N_CORES = 8
B, C = 16384, 2048
B_SHARD = B // N_CORES          # 2048 rows per core
ROW_TILES = B_SHARD // 128      # 16 tiles of 128 rows
EPS = 1e-8

F32 = mybir.dt.float32
F32R = mybir.dt.float32r
LN = mybir.ActivationFunctionType.Ln
MULT = mybir.AluOpType.mult
ADD = mybir.AluOpType.add


def _fix_multi_waits(nc):
    """This walrus build accepts only one sync-wait per instruction, while
    Tile's sem-assigner can attach several. Split extras onto single-wait
    EventSemaphore carriers inserted before the instruction (same engine,
    identical blocking semantics)."""
    for fn in nc.m.functions:
        for blk in fn.blocks:
            insns = list(blk.instructions)
            out = []
            changed = False
            for ins in insns:
                si = ins.sync_info
                waits = list(si.on_wait) if si is not None else []
                if len(waits) > 1:
                    for k, w in enumerate(waits[:-1]):
                        carrier = mybir.InstEventSemaphore(
                            name=f"{ins.name}-wsplit{k}",
                            engine=ins.engine,
                            ins=[],
                            outs=[],
                            sync_info=mybir.SyncInfo(on_wait=[w], on_update=[]),
                        )
                        carrier.debug = ins.debug
                        nc.register_instruction(carrier)
                        out.append(carrier)
                    ins.sync_info = mybir.SyncInfo(
                        on_wait=[waits[-1]], on_update=list(si.on_update)
                    )
                    changed = True
                out.append(ins)
            if changed:
                blk.instructions.clear()
                blk.instructions.extend(out)


class _SlimTailTileContext(tile.TileContext):
    """Skip the second all-engine barrier of the stock kernel tail (the one
    after clear_and_free_semaphores). The final drain already waited for all
    outstanding work and the first barrier aligned the engines; the second
    only guards sem-clears against an engine racing ahead into... nothing
    (end of program). Validated by repeat executions on hardware."""

    def _drain_and_barrier(self, tick_clock, wait_clock):
        from concourse.vector_clock import ScopedClock
        drain_inst = self.nc.sync.drain()
        wait_clock.add_sem_waits(
            drain_inst.ins, ScopedClock({None: tick_clock.global_clock})
        )
        self.nc.all_engine_barrier()
        assert self.sems is not None
        popped = self.nc._tile_sem_poison_stack.pop()
        assert popped is self._sem_poison
        self.nc.clear_and_free_semaphores(list(self.sems.allocated().values()))


def build_kernel(p2_engine="gpsimd", inp_bufs=3, dp_bufs=None, logs_bufs=2, prods_bufs=3, halfp_bufs=4, ps_s_bufs=2, dma_split=4, pipelined_reds=True, acc_onebank=False,
                 s_mode="pe_psum", s_pool_cols=1024, dma_engs=("sync", "sync"),
                 _hack_acc=False, p3b_pool_cols=1248, merged=False, mp_pool_cols=2048,
                 dp_first=False, consts_on_gpsimd=True, slim_tail=False,
                 p2_pool_cols=704, p1_pool_cols=448, ln_preload=False,
                 rt0_d_first=False, last_inline=False, rt0_split_logs=False):
    nc = bass.Bass("TRN2", target_bir_lowering=False, debug=False,
                   num_devices=N_CORES)

    d_ext = nc.dram_tensor("D", [B_SHARD, C], F32R, kind="ExternalInput").ap()
    dp_ext = nc.dram_tensor("Dp", [B_SHARD, C], F32R, kind="ExternalInput").ap()
    ident_ext = nc.dram_tensor("ident", [128, 128], F32R, kind="ExternalInput").ap()
    whalf_ext = nc.dram_tensor("whalf", [128, 1], F32R, kind="ExternalInput").ap()
    wneg_ext = nc.dram_tensor("wneg", [128, 1], F32R, kind="ExternalInput").ap()
    out_ext = nc.dram_tensor("colsum", [1, C], F32, kind="ExternalOutput").ap()

    tc_cls = _SlimTailTileContext if slim_tail else tile.TileContext
    with tc_cls(nc) as tc:
        with tc.tile_pool(name="consts", bufs=1) as consts, \
             tc.tile_pool(name="inp", bufs=inp_bufs) as inp, \
             tc.tile_pool(name="logs", bufs=logs_bufs) as logs, \
             tc.tile_pool(name="prods", bufs=prods_bufs) as prods, \
             tc.tile_pool(name="half", bufs=halfp_bufs) as halfp, \
             tc.tile_pool(name="ps_s", bufs=ps_s_bufs, space="PSUM") as ps_s, \
             tc.tile_pool(name="ps_acc", bufs=1, space="PSUM") as ps_acc:

            c_eng = nc.gpsimd if consts_on_gpsimd else nc.sync
            ident = consts.tile([128, 128], F32R)
            c_eng.dma_start(ident[:], ident_ext[:])
            whalf = consts.tile([128, 1], F32R)
            c_eng.dma_start(whalf[:], whalf_ext[:])
            wneg = consts.tile([128, 1], F32R)
            c_eng.dma_start(wneg[:], wneg_ext[:])
            bias_eps = consts.tile([128, 1], F32)
            nc.vector.memset(bias_eps[:], EPS / 2)
            if ln_preload:
                # tiny Ln so the ACT table set loads while the first input
                # DMAs stream, instead of stalling the first real log
                warm = consts.tile([128, 1], F32)
                nc.scalar.activation(warm[:], bias_eps[:], LN)

            if _hack_acc:
                acc = ps_acc.tile([1, 512], F32)
            elif acc_onebank:
                acc = ps_acc.tile([128, 512], F32)
            else:
                acc = ps_acc.tile([1, C], F32)

            def emit_reds(p1, p2, p3s, first, last):
                if isinstance(p3s, tuple) and p3s and p3s[0] == "four":
                    _, p3a, p3b = p3s
                    for cc in range(4):
                        cs = slice(cc * 512, (cc + 1) * 512)
                        nc.tensor.matmul(acc[:, cs], whalf[:], p1[:, cs],
                                         start=first, stop=False)
                        nc.tensor.matmul(acc[:, cs], whalf[:], p2[:, cs],
                                         start=False, stop=False)
                        nc.tensor.matmul(acc[:, cs], wneg[:], p3a[:, cs],
                                         start=False, stop=False)
                        nc.tensor.matmul(acc[:, cs], wneg[:], p3b[:, cs],
                                         start=False, stop=last)
                    return
                for h in range(2):
                    p3 = p3s[h]
                    for q in range(2):
                        cc = 2 * h + q
                        cs = slice(h * 1024 + q * 512, h * 1024 + (q + 1) * 512)
                        ps = slice(q * 512, (q + 1) * 512)
                        if _hack_acc:
                            acc_out = acc[:, :]
                            tp = None
                        elif acc_onebank:
                            acc_out = acc[32 * cc:32 * cc + 1, :]
                            tp = (0, 32 * cc)
                        else:
                            acc_out = acc[:, cs]
                            tp = None
                        nc.tensor.matmul(acc_out, whalf[:], p1[:, cs],
                                         start=first, stop=False,
                                         tile_position=tp)
                        nc.tensor.matmul(acc_out, whalf[:], p2[:, cs],
                                         start=False, stop=False,
                                         tile_position=tp)
                        nc.tensor.matmul(acc_out, wneg[:], p3[:, ps],
                                         start=False, stop=last,
                                         tile_position=tp)

            pending = None
            for rt in range(ROW_TILES):
                rows = slice(rt * 128, (rt + 1) * 128)
                if merged:
                    # One [128, 2C] tile: D in cols [0, C), D_pred in [C, 2C).
                    # One Ln covers both logs; one product op pair splits at
                    # an arbitrary column for DVE/Pool balance.
                    ddp = inp.tile([128, 2 * C], F32R, tag="ddp")
                    cw = C // dma_split
                    for k in range(dma_split):
                        sl = slice(k * cw, (k + 1) * cw)
                        sl2 = slice(C + k * cw, C + (k + 1) * cw)
                        nc.sync.dma_start(ddp[:, sl], d_ext[rows, sl])
                        nc.sync.dma_start(ddp[:, sl2], dp_ext[rows, sl])
                    log_ddp = logs.tile([128, 2 * C], F32, tag="logddp")
                    nc.scalar.activation(log_ddp[:], ddp[:].bitcast(F32), LN)
                    pm = prods.tile([128, 2 * C], F32R, tag="pm")
                    mx = 2 * C - mp_pool_cols
                    if mx > 0:
                        nc.vector.tensor_tensor(pm[:, :mx],
                                                ddp[:, :mx].bitcast(F32),
                                                log_ddp[:, :mx], MULT)
                    if mx < 2 * C:
                        nc.gpsimd.tensor_tensor(pm[:, mx:],
                                                ddp[:, mx:].bitcast(F32),
                                                log_ddp[:, mx:], MULT)
                    d_t = ddp[:, :C]
                    dp_t = ddp[:, C:]
                    p1 = pm[:, :C]
                    p2 = pm[:, C:]
                else:
                    d_t = inp.tile([128, C], F32R, tag="d")
                    dp_t = inp.tile([128, C], F32R, tag="dp",
                                    bufs=dp_bufs or inp_bufs)
                    cw = C // dma_split
                    eng_d = getattr(nc, {"sync": "sync", "scalar": "scalar",
                                         "vector": "vector"}[dma_engs[0]])
                    eng_dp = getattr(nc, {"sync": "sync", "scalar": "scalar",
                                          "vector": "vector"}[dma_engs[1]])
                    if rt0_d_first and rt == 0:
                        for k in range(dma_split):
                            sl = slice(k * cw, (k + 1) * cw)
                            eng_d.dma_start(d_t[:, sl], d_ext[rows, sl])
                        for k in range(dma_split):
                            sl = slice(k * cw, (k + 1) * cw)
                            eng_dp.dma_start(dp_t[:, sl], dp_ext[rows, sl])
                    else:
                        for k in range(dma_split):
                            sl = slice(k * cw, (k + 1) * cw)
                            eng_d.dma_start(d_t[:, sl], d_ext[rows, sl])
                            eng_dp.dma_start(dp_t[:, sl], dp_ext[rows, sl])

                    p2_eng = nc.gpsimd if p2_engine == "gpsimd" else nc.vector
                    if rt0_split_logs and rt == 0 and not dp_first:
                        # first tile only: half-width logs so ACT starts as
                        # soon as the first half of D has landed
                        log_d = logs.tile([128, C], F32, tag="logd")
                        nc.scalar.activation(log_d[:, :C // 2],
                                             d_t[:, :C // 2].bitcast(F32), LN)
                        nc.scalar.activation(log_d[:, C // 2:],
                                             d_t[:, C // 2:].bitcast(F32), LN)
                        log_dp = logs.tile([128, C], F32, tag="logdp")
                        nc.scalar.activation(log_dp[:, :C // 2],
                                             dp_t[:, :C // 2].bitcast(F32), LN)
                        nc.scalar.activation(log_dp[:, C // 2:],
                                             dp_t[:, C // 2:].bitcast(F32), LN)
                        p1 = prods.tile([128, C], F32R, tag="p1")
                        pc1 = p1_pool_cols if p2_engine == "gpsimd" else 0
                        if pc1 > 0:
                            nc.gpsimd.tensor_tensor(p1[:, :pc1],
                                                    d_t[:, :pc1].bitcast(F32),
                                                    log_d[:, :pc1], MULT)
                        if pc1 < C:
                            nc.vector.tensor_tensor(p1[:, pc1:],
                                                    d_t[:, pc1:].bitcast(F32),
                                                    log_d[:, pc1:], MULT)
                        p2 = prods.tile([128, C], F32R, tag="p2")
                        pc = p2_pool_cols if p2_engine == "gpsimd" else 0
                        if pc > 0:
                            nc.gpsimd.tensor_tensor(p2[:, :pc],
                                                    dp_t[:, :pc].bitcast(F32),
                                                    log_dp[:, :pc], MULT)
                        if pc < C:
                            nc.vector.tensor_tensor(p2[:, pc:],
                                                    dp_t[:, pc:].bitcast(F32),
                                                    log_dp[:, pc:], MULT)
                    elif dp_first:
                        # logDp first so the slow Pool product starts ~1.9us
                        # earlier; p1 is emitted after the h-loop so DVE's
                        # in-order stream never stalls waiting for logD.
                        log_dp = logs.tile([128, C], F32, tag="logdp")
                        nc.scalar.activation(log_dp[:], dp_t[:].bitcast(F32),
                                             LN)
                        p2 = prods.tile([128, C], F32R, tag="p2")
                        pc = p2_pool_cols if p2_engine == "gpsimd" else 0
                        if pc > 0:
                            nc.gpsimd.tensor_tensor(p2[:, :pc],
                                                    dp_t[:, :pc].bitcast(F32),
                                                    log_dp[:, :pc], MULT)
                        if pc < C:
                            nc.vector.tensor_tensor(p2[:, pc:],
                                                    dp_t[:, pc:].bitcast(F32),
                                                    log_dp[:, pc:], MULT)
                        log_d = logs.tile([128, C], F32, tag="logd")
                        nc.scalar.activation(log_d[:], d_t[:].bitcast(F32), LN)
                        p1 = None
                    else:
                        log_d = logs.tile([128, C], F32, tag="logd")
                        nc.scalar.activation(log_d[:], d_t[:].bitcast(F32), LN)
                        log_dp = logs.tile([128, C], F32, tag="logdp")
                        nc.scalar.activation(log_dp[:], dp_t[:].bitcast(F32),
                                             LN)

                        p1 = prods.tile([128, C], F32R, tag="p1")
                        pc1 = p1_pool_cols if p2_engine == "gpsimd" else 0
                        if pc1 > 0:
                            nc.gpsimd.tensor_tensor(p1[:, :pc1],
                                                    d_t[:, :pc1].bitcast(F32),
                                                    log_d[:, :pc1], MULT)
                        if pc1 < C:
                            nc.vector.tensor_tensor(p1[:, pc1:],
                                                    d_t[:, pc1:].bitcast(F32),
                                                    log_d[:, pc1:], MULT)
                        p2 = prods.tile([128, C], F32R, tag="p2")
                        pc = p2_pool_cols if p2_engine == "gpsimd" else 0
                        if pc > 0:
                            nc.gpsimd.tensor_tensor(p2[:, :pc],
                                                    dp_t[:, :pc].bitcast(F32),
                                                    log_dp[:, :pc], MULT)
                        if pc < C:
                            nc.vector.tensor_tensor(p2[:, pc:],
                                                    dp_t[:, pc:].bitcast(F32),
                                                    log_dp[:, pc:], MULT)

                if s_mode == "four_prod":
                    logm = halfp.tile([128, C], F32, tag="logm")
                    for h in range(2):
                        s_ps = ps_s.tile([128, 1024], F32, tag="s")
                        for q in range(2):
                            cs = slice(h * 1024 + q * 512,
                                       h * 1024 + (q + 1) * 512)
                            ps = slice(q * 512, (q + 1) * 512)
                            nc.tensor.matmul(s_ps[:, ps], ident[:], d_t[:, cs],
                                             start=True, stop=False)
                            nc.tensor.matmul(s_ps[:, ps], ident[:],
                                             dp_t[:, cs], start=False,
                                             stop=True)
                        nc.scalar.activation(logm[:, h * 1024:(h + 1) * 1024],
                                             s_ps[:], LN,
                                             scale=0.5, bias=bias_eps[:])
                    p3a = prods.tile([128, C], F32R, tag="p3a")
                    nc.vector.tensor_tensor(p3a[:], d_t[:].bitcast(F32),
                                            logm[:], MULT)
                    p3b = prods.tile([128, C], F32R, tag="p3b")
                    pc = p3b_pool_cols
                    if pc > 0:
                        nc.gpsimd.tensor_tensor(p3b[:, :pc],
                                                dp_t[:, :pc].bitcast(F32),
                                                logm[:, :pc], MULT)
                    if pc < C:
                        nc.vector.tensor_tensor(p3b[:, pc:],
                                                dp_t[:, pc:].bitcast(F32),
                                                logm[:, pc:], MULT)
                    p3s = ("four", p3a, p3b)
                elif s_mode == "split_sbuf":
                    s_t = halfp.tile([128, C], F32, tag="s")
                    pc = s_pool_cols
                    if pc > 0:
                        nc.gpsimd.tensor_tensor(
                            s_t[:, :pc], d_t[:, :pc].bitcast(F32),
                            dp_t[:, :pc].bitcast(F32), ADD)
                    if pc < C:
                        nc.vector.tensor_tensor(
                            s_t[:, pc:], d_t[:, pc:].bitcast(F32),
                            dp_t[:, pc:].bitcast(F32), ADD)
                    logm = halfp.tile([128, C], F32, tag="logm")
                    nc.scalar.activation(logm[:], s_t[:], LN,
                                         scale=0.5, bias=bias_eps[:])
                    p3 = halfp.tile([128, C], F32R, tag="p3")
                    nc.vector.tensor_tensor(p3[:], s_t[:], logm[:], MULT)
                    p3s = [p3[:, :1024], p3[:, 1024:]]
                else:
                    p3s = []
                    for h in range(2):
                        s_ps = ps_s.tile([128, 1024], F32, tag="s")
                        for q in range(2):
                            cs = slice(h * 1024 + q * 512,
                                       h * 1024 + (q + 1) * 512)
                            ps = slice(q * 512, (q + 1) * 512)
                            nc.tensor.matmul(s_ps[:, ps], ident[:], d_t[:, cs],
                                             start=True, stop=False)
                            nc.tensor.matmul(s_ps[:, ps], ident[:],
                                             dp_t[:, cs], start=False,
                                             stop=True)

                        logm = halfp.tile([128, 1024], F32, tag="logm")
                        nc.scalar.activation(logm[:], s_ps[:], LN,
                                             scale=0.5, bias=bias_eps[:])
                        p3 = halfp.tile([128, 1024], F32R, tag="p3")
                        nc.vector.tensor_tensor(p3[:], s_ps[:], logm[:], MULT)
                        p3s.append(p3)
                        if (last_inline and pipelined_reds
                                and rt == ROW_TILES - 1):
                            for q in range(2):
                                cs = slice(h * 1024 + q * 512,
                                           h * 1024 + (q + 1) * 512)
                                ps = slice(q * 512, (q + 1) * 512)
                                nc.tensor.matmul(acc[:, cs], whalf[:],
                                                 p1[:, cs], start=False,
                                                 stop=False)
                                nc.tensor.matmul(acc[:, cs], whalf[:],
                                                 p2[:, cs], start=False,
                                                 stop=False)
                                nc.tensor.matmul(acc[:, cs], wneg[:],
                                                 p3[:, ps], start=False,
                                                 stop=True)

                if not merged and dp_first and p1 is None:
                    p1 = prods.tile([128, C], F32R, tag="p1")
                    nc.vector.tensor_tensor(p1[:], d_t[:].bitcast(F32),
                                            log_d[:], MULT)

                if not pipelined_reds:
                    emit_reds(p1, p2, p3s, rt == 0, rt == ROW_TILES - 1)
                else:
                    if pending is not None:
                        emit_reds(pending[0], pending[1], pending[2],
                                  pending[3], False)
                    pending = (p1, p2, p3s, rt == 0)
            if pipelined_reds and pending is not None:
                emit_reds(pending[0], pending[1], pending[2],
                          pending[3], not last_inline)

            if acc_onebank:
                out_sb = consts.tile([128, 512], F32)
                for cc in range(4):
                    pr = slice(32 * cc, 32 * cc + 1)
                    nc.vector.tensor_copy(out_sb[pr, :], acc[pr, :])
                    nc.sync.dma_start(
                        out_ext[0:1, cc * 512:(cc + 1) * 512], out_sb[pr, :])
            else:
                out_sb = consts.tile([1, C], F32)
                for q in range(4):
                    ps = slice(q * 512, (q + 1) * 512)
                    src_ps = slice(0, 512) if _hack_acc else ps
                    # split the single-partition PSUM->SBUF copies across
                    # DVE and ACT so they drain in parallel at kernel tail
                    eng = nc.vector if q % 2 == 0 else nc.scalar
                    if eng is nc.vector:
                        eng.tensor_copy(out_sb[:, ps], acc[:, src_ps])
                    else:
                        nc.scalar.copy(out_sb[:, ps], acc[:, src_ps])
                nc.sync.dma_start(out_ext[:], out_sb[:])

    _fix_multi_waits(nc)
    return nc


_NC_CACHE = None


def _get_nc():
    global _NC_CACHE
    if _NC_CACHE is None:
        _NC_CACHE = build_kernel()
    return _NC_CACHE


def make_in_maps(D, D_pred):
    D = np.ascontiguousarray(np.asarray(D, dtype=np.float32))
    D_pred = np.ascontiguousarray(np.asarray(D_pred, dtype=np.float32))
    ident = np.eye(128, dtype=np.float32)
    whalf = np.full((128, 1), 0.5, dtype=np.float32)
    wneg = np.full((128, 1), -0.5, dtype=np.float32)
    in_maps = []
    for c in range(N_CORES):
        rows = slice(c * B_SHARD, (c + 1) * B_SHARD)
        in_maps.append({
            "D": D[rows],
            "Dp": D_pred[rows],
            "ident": ident,
            "whalf": whalf,
            "wneg": wneg,
        })
    return in_maps


def finish(results):
    colsum = np.zeros(C, dtype=np.float64)
    for c in range(N_CORES):
        colsum += results[c]["colsum"][0].astype(np.float64)
    w = (C - np.arange(C)).astype(np.float64)
    return np.asarray((colsum * w).sum() / B, dtype=np.float32)


def kernel(D, D_pred):
    nc = _get_nc()
    res = run_bass_kernel_spmd(nc, make_in_maps(D, D_pred), list(range(N_CORES)))
    return finish(res.results)


if __name__ == "__main__":
    rng = np.random.default_rng(0)
    D = rng.uniform(1e-4, 1.0, (B, C)).astype(np.float32)
    Dp = rng.uniform(1e-4, 1.0, (B, C)).astype(np.float32)
    out = kernel(D, Dp)
    print("kernel out:", out)
